# revision 29
# baseline (speedup 1.0000x reference)
"""Trainium2 Bass kernel for nn_MaxPool_730144440853.

Math (per batch b):
    d = einsum("czn,dc->dzn", x[b], W)
    scores[c, n] = sum_z x[b,c,z,n] * d[b,c,z,n]
    idx[c] = argmax_n scores[c, n]
    out[b, c, :] = x[b, c, :, idx[c]]

Sharding: data-parallel over batch B=8 across the 8 NeuronCores; W replicated.

Device pipeline (per core, fp16 inputs):
  - PE: d = W @ x per (n-tile, half, z), fp16 matmuls into fp32 PSUM,
    k-major so 3 consecutive matmuls share stationary weights.
  - Act: cast d PSUM fp32 -> SBUF fp16 (unlocks the DVE 2x 16-bit mode).
  - DVE (2x fp16), instructions merged across the two channel halves:
    p = x * d (FD 3072), one fused strided-slab add for both z-adds
    (FD 2048), and a running elementwise fold m[h, q] = max_t s[h, t*T+q]
    (FD 1024).
  - Tail: the folded max arrays (2 x [128, 512] fp16, 256KB) are DMA'd out
    raw; the host takes top-K per row (no device max8/max_index).

Host: expand the top-K folded positions q_j to K*16 candidate n's,
re-score them exactly in float64 from the original fp32 inputs, and pick
the argmax (ties toward smallest n, matching jnp.argmax first-occurrence).
"""

import sys

sys.path.insert(0, "/opt/trn_rl_repo")

import numpy as np

B, C, Z, N = 8, 256, 3, 8192
H = C // 128  # partition halves (2)
T = 512  # n-tile width
NT = N // T
ZT = Z * T
TOPK = 16  # folded positions expanded per row on the host

_cache = {}


def _split_multiwait_bir(bir_json: bytes) -> bytes:
    """walrus in this toolchain rejects instructions carrying more than one
    semaphore wait ("Too many sync wait commands"). Rewrite the BIR so any
    instruction with >1 on_wait keeps only the last one; the others are
    hoisted into single-wait EventSemaphore instructions inserted just
    before it on the same engine (engine program order makes this
    equivalent)."""
    import json

    d = json.loads(bir_json)
    n_new = 0
    for fn in d.get("functions", []):
        for blk in fn.get("blocks", []):
            insts = blk.get("instructions", [])
            out = []
            for ins in insts:
                si = ins.get("sync_info")
                waits = si.get("on_wait") if si else None
                if waits and len(waits) > 1:
                    for w in waits[:-1]:
                        out.append(
                            {
                                "debug": ins.get("debug", 0),
                                "engine": ins["engine"],
                                "ins": [],
                                "name": f"{ins['name']}_hw{n_new}",
                                "opcode": "EventSemaphore",
                                "outs": [],
                                "sync_info": {"on_update": [], "on_wait": [w]},
                            }
                        )
                        n_new += 1
                    si["on_wait"] = [waits[-1]]
                out.append(ins)
            blk["instructions"] = out
    return json.dumps(d).encode()


def _apply_tile_patch():
    """Install the multi-wait splitter in front of walrus compilation."""
    from concourse import bass_utils, bass2jax

    if getattr(bass_utils, "_ant_split_multiwait", False):
        return

    orig = bass_utils.compile_bir_kernel

    def patched(bir_json, tmpdir, neff_name="file.neff"):
        return orig(_split_multiwait_bir(bir_json), tmpdir, neff_name=neff_name)

    bass_utils.compile_bir_kernel = patched
    bass2jax.compile_bir_kernel = patched
    bass_utils._ant_split_multiwait = True


def _build_nc():
    import concourse.bass as bass
    import concourse.mybir as mybir
    from concourse.tile import TileContext

    _apply_tile_patch()

    f16 = mybir.dt.float16
    f32 = mybir.dt.float32
    add = mybir.AluOpType.add
    mult = mybir.AluOpType.mult
    vmax = mybir.AluOpType.max

    nc = bass.Bass(target_bir_lowering=False)
    # x{k}[t] = fp16 tile [128, Z*T], channels k*128..k*128+127, n-tile t.
    x0 = nc.dram_tensor("x0", [NT, 128, ZT], f16, kind="ExternalInput")
    x1 = nc.dram_tensor("x1", [NT, 128, ZT], f16, kind="ExternalInput")
    # wt{k}[c_in - k*128, c_out] = W[c_out, c_in]; lhsT slices for the PE.
    wt0 = nc.dram_tensor("wt0", [128, C], f16, kind="ExternalInput")
    wt1 = nc.dram_tensor("wt1", [128, C], f16, kind="ExternalInput")
    # folded max arrays: mf[p, h, q] = max_t s[h*128+p, t*T+q]
    mf = nc.dram_tensor("mf", [128, H, T], f16, kind="ExternalOutput")

    xsrc = (x0, x1)

    with TileContext(nc) as tc:
        with (
            tc.tile_pool(name="wts", bufs=1) as wpool,
            tc.tile_pool(name="xin", bufs=6) as xpool,
            tc.tile_pool(name="dcast", bufs=4) as cpool,
            tc.tile_pool(name="prod", bufs=3) as ppool,
            tc.tile_pool(name="fold", bufs=2) as mpool,
            tc.tile_pool(name="psum", bufs=2, space="PSUM") as dpool,
        ):
            # Pipeline head: the first matmul needs x0-z0, x1-z0 and the
            # weights; dispatch those DMAs first, fine-grained, then the
            # rest of tile 0.
            xb0 = xpool.tile([128, 2, Z, T], f16, tag="xb", name="xb0")
            wt_sb = [
                wpool.tile([128, C], f16, tag=f"wt{k}", name=f"wt_sb{k}")
                for k in range(2)
            ]
            # interleave so the first matmul's inputs (wt0 + x0-z0) complete
            # after ~192KB of DMA instead of queueing wt0 behind 448KB of x
            nc.sync.dma_start(
                out=xb0[:, 0, 0, 0 : T // 2], in_=x0[0, :, 0 : T // 2]
            )
            nc.sync.dma_start(out=wt_sb[0][:], in_=wt0[:])
            nc.sync.dma_start(
                out=xb0[:, 0, 0, T // 2 : T], in_=x0[0, :, T // 2 : T]
            )
            nc.sync.dma_start(out=xb0[:, 1, 0, :], in_=x1[0, :, 0:T])
            nc.sync.dma_start(out=wt_sb[1][:], in_=wt1[:])

            for z in range(1, Z):
                for k in range(2):
                    nc.sync.dma_start(
                        out=xb0[:, k, z, :], in_=xsrc[k][0, :, z * T : (z + 1) * T]
                    )

            # running folded max, ping-pong buffers via the pool
            # (memset on the otherwise-idle GpSimd engine, off the DVE queue)
            m_prev = mpool.tile([128, 2, T], f16, tag="m", name="m0")
            nc.gpsimd.memset(m_prev[:], -60000.0)

            for t in range(NT):
                if t == 0:
                    xb = xb0
                elif t == 1:
                    # fine-grained loads: early consumers wait only for
                    # their z-slice, not the whole 786KB tile
                    xb = xpool.tile([128, 2, Z, T], f16, tag="xb", name="xbh")
                    for z in range(Z):
                        for k in range(2):
                            nc.sync.dma_start(
                                out=xb[:, k, z, :],
                                in_=xsrc[k][t, :, z * T : (z + 1) * T],
                            )
                else:
                    xb = xpool.tile([128, 2, Z, T], f16, tag="xb", name="xb")
                    for k in range(2):
                        nc.sync.dma_start(out=xb[:, k], in_=xsrc[k][t])

                dc = cpool.tile([128, 2, Z, T], f16, tag="dc", name="dc")
                p = ppool.tile([128, 2, 5, T], f16, tag="p", name="p")
                for h in range(H):
                    d = dpool.tile([128, Z, T], f32, name="d_psum")
                    if t == 0:
                        # Pipeline-fill special case: z-major matmuls with
                        # per-z cast and multiply, so DVE/Act work starts
                        # after 2 matmuls + a 512-el copy instead of a full
                        # 6-matmul group + 1536-el copy.
                        for z in range(Z):
                            for k in range(2):
                                nc.tensor.matmul(
                                    d[:, z, :],
                                    wt_sb[k][:, h * 128 : (h + 1) * 128],
                                    xb[:, k, z, :],
                                    start=(k == 0),
                                    stop=(k == 1),
                                )
                            nc.scalar.copy(dc[:, h, z, :], d[:, z, :])
                            nc.vector.tensor_tensor(
                                p[:, h, z, :], xb[:, h, z, :], dc[:, h, z, :], op=mult
                            )
                    else:
                        # k-major order: 3 consecutive matmuls share the same
                        # stationary weights (one PSUM bank = 512 fp32 per mm).
                        for k in range(2):
                            for z in range(Z):
                                nc.tensor.matmul(
                                    d[:, z, :],
                                    wt_sb[k][:, h * 128 : (h + 1) * 128],
                                    xb[:, k, z, :],
                                    start=(k == 0),
                                    stop=(k == 1),
                                )
                        nc.scalar.copy(dc[:, h], d[:])
                if t > 0:
                    # Merged across halves: one 3072-elem fp16 2x multiply.
                    nc.vector.tensor_tensor(
                        p[:, :, 0:3, :], xb[:, :, :, :], dc[:], op=mult
                    )
                # Fused strided-slab add for both halves: per half,
                # slab 3 = s1 = p0+p1, slab 4 = s = s1+p2 (slab 4 reads the
                # s1 values slab 3 wrote 512 elements earlier in the stream).
                nc.vector.tensor_tensor(
                    p[:, :, 3:5, :], p[:, :, 0:4:3, :], p[:, :, 1:3, :], op=add
                )
                m_new = mpool.tile([128, 2, T], f16, tag="m", name="mn")
                if t < NT - 1:
                    nc.vector.tensor_tensor(
                        m_new[:], m_prev[:], p[:, :, 4, :], op=vmax
                    )
                else:
                    # split the final fold per half so the first 128KB of
                    # output DMA overlaps the second half's max
                    nc.vector.tensor_tensor(
                        m_new[:, 0], m_prev[:, 0], p[:, 0, 4, :], op=vmax
                    )
                    nc.sync.dma_start(out=mf[:, 0], in_=m_new[:, 0])
                    nc.vector.tensor_tensor(
                        m_new[:, 1], m_prev[:, 1], p[:, 1, 4, :], op=vmax
                    )
                    nc.sync.dma_start(out=mf[:, 1], in_=m_new[:, 1])
                m_prev = m_new

    return nc


def _get_nc():
    if "nc" not in _cache:
        _cache["nc"] = _build_nc()
    return _cache["nc"]


def _make_in_maps(x, W):
    """Per-core input dict: fp16 tiled x halves + transposed fp16 W slices."""
    wt = np.ascontiguousarray(W.T).astype(np.float16)
    x16 = x.astype(np.float16)  # [B, C, Z, N]
    in_maps = []
    for b in range(B):
        m = {"wt0": wt[:128], "wt1": wt[128:]}
        for k in range(2):
            # [128, Z, NT, T] -> [NT, 128, Z*T]
            xk = x16[b, k * 128 : (k + 1) * 128].reshape(128, Z, NT, T)
            m[f"x{k}"] = np.ascontiguousarray(xk.transpose(2, 0, 1, 3)).reshape(
                NT, 128, ZT
            )
        in_maps.append(m)
    return in_maps


def _run_device(x, W):
    from concourse.bass_utils import run_bass_kernel_spmd

    nc = _get_nc()
    res = run_bass_kernel_spmd(nc, _make_in_maps(x, W), core_ids=list(range(B)))
    # mf[p, h, q] -> m[b, h*128+p, q]
    mflat = np.stack(
        [
            r["mf"].reshape(128, H, T).transpose(1, 0, 2).reshape(C, T)
            for r in res.results
        ]
    )  # [B, C, T] f16
    return mflat, res


def _host_finalize(x, W, mfold):
    """Expand the top-K folded positions per row to K*NT candidate indices,
    re-score them exactly in float64, and gather the winning 3-vector."""
    out = np.empty((B, C, Z), dtype=x.dtype)
    W64 = W.astype(np.float64)
    offs = (np.arange(NT, dtype=np.int64) * T)[None, :, None]  # [1, NT, 1]
    NC = NT * TOPK
    # top-K folded positions per row
    qk = np.argpartition(-mfold.astype(np.float32), TOPK, axis=-1)[
        ..., :TOPK
    ]  # [B, C, K]
    for b in range(B):
        xb = x[b]  # [C, Z, N] fp32
        q = qk[b].astype(np.int64)  # [C, K]
        I = (q[:, None, :] + offs).reshape(C, NC)  # [C, NC]
        xb64 = xb.astype(np.float64)
        s_cand = np.empty((C, NC), dtype=np.float64)
        blk = 64
        for r0 in range(0, C, blk):
            r1 = r0 + blk
            # cols[c_in, z, r, j] = x[b, c_in, z, I[r, j]]
            cols = xb64[:, :, I[r0:r1]]  # [C, Z, blk, NC]
            d_cand = np.einsum("rc,czrj->rzj", W64[r0:r1], cols)
            xr = np.take_along_axis(
                xb64[r0:r1], I[r0:r1, None, :], axis=2
            )  # [blk, Z, NC]
            s_cand[r0:r1] = (xr * d_cand).sum(axis=1)
        # argmax over candidates; break exact ties toward the smallest n
        # (matches jnp.argmax first-occurrence semantics).
        order = np.lexsort((I, -s_cand), axis=1)
        jbest = order[:, 0]
        nbest = I[np.arange(C), jbest]
        out[b] = np.take_along_axis(xb, nbest[:, None, None], axis=2)[:, :, 0]
    return out


def kernel(x, W):
    x = np.asarray(x, dtype=np.float32)
    W = np.asarray(W, dtype=np.float32)
    mfold, _ = _run_device(x, W)
    return _host_finalize(x, W, mfold)
